# revision 27
# baseline (speedup 1.0000x reference)
"""Cost-volume layer (17-shift cross pattern, R=4) for Trainium2, 8 NeuronCores.

out[b,s,h,w] = sum_c src[b,c,h,w] * tgt[b,c,h+dh_s,w+dw_s]   (tgt zero-padded)

Strategy (column-progressive pipeline)
--------------------------------------
Shard: 8 cores = batch(4) x H-halves(2). Per core the inputs are HOST-
TRANSPOSED to w-major: src [C, W=160, 48], tgt [C, 160, 56] (8-row H halo,
W pad added on device via memset strips). C=128 is the SBUF partition dim,
contracted on the TensorEngine via banded correlations:

- vertical shifts  (dh=-4..4): per column w, matmul
    src[:, w, :]^T @ tgt[:, w+4, :]  ->  [48, 56] band
- horizontal shifts (dw=-4..4): per row h, per 32-col chunk i, matmul
    src[:, 32i:32i+32, h]^T @ tgt[:, 32i:32i+40, h+4] -> [32, 40] band

Streaming: tgt rides the sync HWDGE ring, src the scalar(ACT) ring, FIVE
transfers each (8/40/48/48/16 cols) -- the HWDGE admits only ~5
outstanding dma_starts per ring before it throttles on per-transfer
completion semaphores (~0.9us each), so 5 big transfers keep all 16 DMA
queues at the HBM rate.  V bank p unlocks column-by-column via Tile
subtile tracking; H bank i is issued right behind the piece completing
it.  PSUM->SBUF copies alternate DVE/ACT; band flushes ride the gpsimd
SWDGE (V banks in pairs, [112, 896] with hole partitions 48:64 along for
the ride).  All 5 V-pair stage buffers and 5 H stage buffers are live at
once so no copy ever backpressures on a flush.

p-state keeper: the PE reaches its full 2.4GHz clock only after ~3us of
continuous execution (1.2GHz before that); stream-paced idle gaps would
keep it at 1.2GHz the whole run.  Wide dummy matmuls (448-row rhs over a
memset scratch operand, written to a never-read PSUM bank) are woven
between the early banks so the clock ramps by ~12us and the whole
back-half runs at 2x.  Short gaps only pause, not reset, the ramp.

Diagonals are gathered host-side from the bands (pure indexing).
"""
import numpy as np
from contextlib import ExitStack

import concourse.bacc as bacc
import concourse.tile as tile
from concourse import mybir
from concourse import bass_utils

R = 4
B, C, H, W = 4, 128, 96, 160
NCORES = 8
HSH = H // 2            # 48 output rows per shard
HT = HSH + 2 * R        # 56 tgt rows (with halo)
WT = W + 2 * R          # 168 padded width (device)
F32 = mybir.dt.float32
F16 = mybir.dt.float16

PC = 16                 # piece width (cols)
NP = W // PC            # 10 pieces
# vertical: bank p covers w in [16p, 16p+16); w = 16p + 8g + s,
# g in {0,1} -> PSUM partition base 64g (holes 48:64), s in 0..8
VS = 8                  # slots per group
# horizontal: bank i covers chunks (h, i), h in 0..47; partition base
# 32*(h%4), slot h//4 in 0..11
MH = 32
NH = MH + 2 * R         # 40
NCH = W // MH           # 5 chunk columns = 5 H banks
HSLOT = HSH // 4        # 12

SHIFTS = [(0, 0)]
for i in range(1, R + 1):
    SHIFTS.extend([(-i, 0), (i, 0), (0, -i), (0, i)])


def build_nc():
    nc = bacc.Bacc("TRN2", target_bir_lowering=False)
    src = nc.dram_tensor("src", [C, W * HSH], F16, kind="ExternalInput")
    tgt = nc.dram_tensor("tgt", [C, W * HT], F16, kind="ExternalInput")
    # vband[gi] = V banks {2gi,2gi+1}: [112, 2, 8, 56] (partition 64g+hh)
    vband = nc.dram_tensor("vband", [NP // 2, 112, 2 * VS * HT], F16,
                           kind="ExternalOutput")
    # hband[i] = H bank i: [128, 12, 40] (partition 32*(h%4)+m)
    hband = nc.dram_tensor("hband", [NCH, 128, HSLOT * NH], F16,
                           kind="ExternalOutput")

    with ExitStack() as ctx:
        tc = ctx.enter_context(tile.TileContext(nc))
        ins = ctx.enter_context(tc.tile_pool(name="ins", bufs=1))
        psum = ctx.enter_context(tc.tile_pool(name="psum", bufs=1, space="PSUM"))
        stage = ctx.enter_context(tc.tile_pool(name="stage", bufs=1))

        src_sb = ins.tile([C, W * HSH], F16)
        tgt_sb = ins.tile([C, WT * HT], F16)
        src3 = src_sb.rearrange("c (w h) -> c w h", h=HSH)
        tgt3 = tgt_sb.rearrange("c (w r) -> c w r", r=HT)
        srcd = src[:].rearrange("c (w h) -> c w h", h=HSH)
        tgtd = tgt[:].rearrange("c (w r) -> c w r", r=HT)

        # zero the W-pad strips (cols 0:4 and 164:168 of padded tgt)
        nc.gpsimd.memset(tgt3[:, 0:R, :], 0.0)
        nc.gpsimd.memset(tgt3[:, R + W:WT, :], 0.0)
        # scratch operand for the p-state keeper matmuls: memset so the
        # dummies depend on nothing but this cheap early memset
        wsrc = ins.tile([C, 448], F16, name="warm_src")
        nc.gpsimd.memset(wsrc[:], 0.0)

        # issue ALL input piece loads up front: tgt on sync, src on scalar.
        # The DMA queues stream them back to back at the HBM rate; compute
        # unlocks per piece via Tile's subtile dependency tracking.  The
        # first 16-col piece of each stream is split in two so the very
        # first matmuls unlock earlier.
        # The HWDGE admits only ~5 outstanding dma_starts per ring; past
        # that, each further D2D waits for a transfer COMPLETION semaphore
        # (~0.9us) -- many small transfers throttle the ring well below the
        # queue rate.  So: exactly 5 transfers per ring, sized small first
        # (fast PE start) and small last (short dependency tail), with the
        # bulk in the middle.  All 5 fit the DGE pipeline, so the 16 DMA
        # queues stream the whole input at the full HBM rate.
        cuts = [0, 8, 48, 96, 144, 160]
        for c0, c1 in zip(cuts[:-1], cuts[1:]):
            nc.sync.dma_start(out=tgt3[:, R + c0:R + c1, :],
                              in_=tgtd[:, c0:c1, :])
        for c0, c1 in zip(cuts[:-1], cuts[1:]):
            nc.scalar.dma_start(out=src3[:, c0:c1, :],
                                in_=srcd[:, c0:c1, :])

        copy_flip = [0]

        def stage_copy(dst, src_ap, eng=None):
            # GPSIMD cannot access PSUM on TRN2 -> DVE/ACT only
            if eng is None:
                eng = copy_flip[0] % 2
                copy_flip[0] += 1
            (nc.vector.tensor_copy, nc.scalar.copy)[eng](out=dst, in_=src_ap)

        vst_hold = [None]
        deferred = []

        # p-state keeper: the PE only reaches its full 2.4GHz clock after
        # ~3us of CONTINUOUS execution; stream-paced gaps keep resetting it
        # to the 1.2GHz mid state.  Wide dummy matmuls on already-resident
        # piece-0 data (written to a never-read scratch PSUM slot) fill the
        # stream gaps so the clock ramps and the real matmuls run ~2x.
        dscr = psum.tile([HSH, 448], F32, tag="dummy", bufs=1)

        def warm(n):
            for _ in range(n):
                nc.tensor.matmul(
                    out=dscr[:],
                    lhsT=wsrc[:, 0:HSH],
                    rhs=wsrc[:],
                    start=True, stop=True,
                    tile_position=(0, 0),
                )

        def vert_bank(p, eng=None):
            pt = psum.tile([112, VS * HT], F32, tag="vp", bufs=4)
            for g in range(2):
                for s in range(VS):
                    w = PC * p + VS * g + s
                    nc.tensor.matmul(
                        out=pt[64 * g:64 * g + HSH, s * HT:(s + 1) * HT],
                        lhsT=src3[:, w, :],
                        rhs=tgt3[:, w + R, :],
                        start=True, stop=True,
                        tile_position=(0, 64 * g),
                    )
            if p % 2 == 0:
                vst_hold[0] = stage.tile([112, 2 * VS * HT], F16, tag="vs",
                                         bufs=5, name=f"vst{p}")
            st = vst_hold[0]
            seg = VS * HT
            half = p % 2
            stage_copy(st[:, half * seg:(half + 1) * seg], pt, eng)
            # flush bank pairs via GPSIMD SWDGE (cheap sequencer dispatch,
            # async Q7 descriptor gen); holes 48:64 ride along.  The first
            # three pairs are DEFERRED to late in the gpsimd stream: their
            # bytes would otherwise enter the shared DMA queues ahead of
            # the last input transfers and delay them; issued late, they
            # drain under the PE tail instead.
            if half == 1:
                if p // 2 <= 2:
                    deferred.append((p // 2, st))
                else:
                    nc.gpsimd.dma_start(out=vband[:][p // 2], in_=st)

        def horiz_bank(i, eng=None, split=False):
            pt = psum.tile([128, HSLOT * NH], F32, tag="hp", bufs=3)
            for h in range(HSH):
                base = 32 * (h % 4)
                j = h // 4
                nc.tensor.matmul(
                    out=pt[base:base + MH, j * NH:(j + 1) * NH],
                    lhsT=src3[:, MH * i:MH * (i + 1), h],
                    rhs=tgt3[:, MH * i:MH * i + NH, h + R],
                    start=True, stop=True,
                    tile_position=(0, base),
                )
            st = stage.tile([128, HSLOT * NH], F16, tag="hs", bufs=5,
                            name=f"hst{i}")
            # H flushes ride the GPSIMD SWDGE queue: ~25ns sequencer
            # dispatch, async Q7 descriptor generation.  The final bank is
            # copied+flushed in two halves so the first half's copy and
            # descriptor gen overlap the second half's matmuls.
            if split:
                mid = (HSLOT // 2) * NH
                stage_copy(st[:, 0:mid], pt[:, 0:mid], eng)
                nc.gpsimd.dma_start(out=hband[:][i][:, 0:mid],
                                    in_=st[:, 0:mid])
                stage_copy(st[:, mid:], pt[:, mid:], eng)
                nc.gpsimd.dma_start(out=hband[:][i][:, mid:],
                                    in_=st[:, mid:])
            else:
                stage_copy(st, pt, eng)
                nc.gpsimd.dma_start(out=hband[:][i], in_=st)

        # schedule: V banks as pieces land; H bank i right after V(2i+2)
        # (it needs tgt cols [32i-4, 32i+36) = pieces up to 2i+2 plus the
        # memset strips).  The run ends V9 -> H4 so the last copy (ACT) and
        # flush (gpsimd, smallest band) pipeline behind V9's (DVE, sync).
        for p in range(NP):
            vert_bank(p, eng=0 if p == NP - 1 else None)
            if p <= 4:
                warm(2)
            if p >= 2 and p % 2 == 0:
                horiz_bank(p // 2 - 1)
                if p <= 4:
                    warm(2)
            if p == NP - 2:
                for gi, st in deferred:
                    nc.gpsimd.dma_start(out=vband[:][gi], in_=st)
        horiz_bank(NCH - 1, eng=1)

    nc.compile()
    return nc


_NC_CACHE = []


def _get_nc():
    if not _NC_CACHE:
        _NC_CACHE.append(build_nc())
    return _NC_CACHE[0]


def shard_inputs(src, tgt):
    src = np.asarray(src, dtype=np.float32)
    tgt = np.asarray(tgt, dtype=np.float32)
    tp = np.pad(tgt, ((0, 0), (0, 0), (R, R), (0, 0)))  # pad H only
    in_maps = []
    for core in range(NCORES):
        b, hh = divmod(core, 2)
        h0 = hh * HSH
        s = src[b, :, h0:h0 + HSH, :].transpose(0, 2, 1)       # [C, W, 48]
        t = tp[b, :, h0:h0 + HT, :].transpose(0, 2, 1)         # [C, W, 56]
        in_maps.append({
            "src": np.ascontiguousarray(s).reshape(C, W * HSH).astype(np.float16),
            "tgt": np.ascontiguousarray(t).reshape(C, W * HT).astype(np.float16),
        })
    return in_maps


def extract_output(results):
    """results: per core 'vband' [10, 96, 448], 'hband' [5, 128, 480]."""
    out = np.zeros((B, len(SHIFTS), H, W), np.float32)
    hidx = np.arange(HSH)
    midx = np.arange(MH)
    for core in range(NCORES):
        b, hh = divmod(core, 2)
        h0 = hh * HSH
        vb = np.asarray(results[core]["vband"]).astype(np.float32)
        vb = vb.reshape(NP // 2, 112, 2, VS, HT)   # [gi, part, half, s, r]
        # part = 64g + hh' (holes 48:64); w = 16*(2gi+half) + 8g + s
        vbg = np.stack([vb[:, 0:HSH], vb[:, 64:64 + HSH]], axis=1)
        # [gi, g, hh', half, s, r] -> [gi, half, g, s, hh', r]
        vbw = vbg.transpose(0, 3, 1, 4, 2, 5).reshape(W, HSH, HT)
        hb = np.asarray(results[core]["hband"]).astype(np.float32)
        hb = hb.reshape(NCH, 4, MH, HSLOT, NH)     # [i, h%4, m, h//4, n]
        hb = hb.transpose(3, 1, 0, 2, 4).reshape(HSH, NCH, MH, NH)
        for si, (dh, dw) in enumerate(SHIFTS):
            if dw == 0:
                v = vbw[:, hidx, hidx + dh + R]        # [W, 48]
                out[b, si, h0:h0 + HSH, :] = v.T
            else:
                v = hb[:, :, midx, midx + dw + R]      # [48, 5, 32]
                out[b, si, h0:h0 + HSH, :] = v.reshape(HSH, W)
    return out


def kernel(src, tgt, **run_kwargs):
    nc = _get_nc()
    in_maps = shard_inputs(src, tgt)
    res = bass_utils.run_bass_kernel_spmd(
        nc, in_maps, core_ids=list(range(NCORES)), **run_kwargs
    )
    out = extract_output(res.results)
    kernel.last_result = res
    return out
